# revision 39
# baseline (speedup 1.0000x reference)
"""Trainium2 Bass kernel for nn_Attentional_Aggregation (segment softmax attention).

Math (reference):
    keys_i = emb_i @ Wk.T + bk
    q_g    = emb[last(g)] @ Wq.T + bq
    logit_i = <q_{g(i)}, keys_i>
    w = segment_softmax(logit)
    out_g = sum_{i in g} w_i * keys_i

Reformulation:
    logit_i = <qk_{g(i)}, emb_i>,  qk_g = embL_g @ (Wq.T Wk) + bq Wk
    out_g = (sum e_i emb_i / sum e_i) @ Wk.T + bk   (device: Wk @ S and denom;
                                                     host: divide + bias)

Device strategy (per core, fully static SPMD program):
  phase A: qkT[c, g] = ARm.T @ embLT (+ u bias via ACT), SBUF-resident, 512-chunks.
  phase B, software-pipelined (stage1(b) || stage2(b-1)) over 98 blocks
  (128 groups, C_b element tiles of 128 each):
    stage1: plain DMAs of natural emb [i, t*129+c] bf16 (col 128 = ones) and
      host-pretransposed embT [c, t*128+i] f16; per tile logits MM
      (lhsT=embT_t, rhs=qkT_blk) into [128, CH*128] PSUM chunks; one EXP per
      chunk -> E bf16 (ACT).
    stage2: one-hot ohb = is_eq(iotab, srel bcast) + one batched multiply
      me = ohb*E (DVE); flip-scatter: spT[g, 0:129] += me_t.T @ [emb_t | 1]
      accumulates BOTH the numerator S_T[g,c] and the denominator (col 128)
      in a single MM chain per tile; den col + S_T strip copied out (DVE/ACT).
  Every 32 blocks: ONE batched SBUF->SBUF xbar transpose turns the S_T strips
  into S [c, b, g] (few xbar-mode toggles - per-block transposes serialize the
  DMA pipeline), then projections otp = Wk @ S batched 4 blocks per weight
  load; bf16 outT. Host: divide by den, add bk, un-permute groups.

Sharding: 12500 groups per core; groups bin-packed (snake by size) into 98
blocks of <=128 groups to equalize per-block element counts; per-block tile
counts C_b shared across cores (max profile). Host un-permutes the output.
"""

import os
import numpy as np
import ml_dtypes

import concourse.bacc as bacc
import concourse.bass as bass
import concourse.mybir as mybir
import concourse.tile as tile
from concourse.bass_utils import run_bass_kernel_spmd

BF16 = ml_dtypes.bfloat16
FP16 = np.float16

N = 1_000_000
G = 100_000
D = 128
NCORES = 8
NBLK = 98
GPC = G // NCORES          # groups per core (12500)
GC = NBLK * 128            # group slots per core (12544)

# Exposed for test harness
LAST_EXEC_NS = None
LAST_RESULTS = None

_cache = {}


def _build_program(C, ncores=NCORES, enable_asserts=False):
    """Build the SPMD Bass program. C = per-block tile counts (len NBLK)."""
    C = list(C)
    nblk = len(C)
    Cmax = max(C)
    assert Cmax <= 12, f"PSUM budget assumes Cmax<=12, got {Cmax}"
    tot = sum(C)
    f32 = mybir.dt.float32
    bf16 = mybir.dt.bfloat16
    f16 = mybir.dt.float16
    ts = bass.ts
    gc = nblk * 128

    nc = bacc.Bacc(
        "TRN2",
        target_bir_lowering=False,
        debug=False,
        enable_asserts=enable_asserts,
        num_devices=ncores,
    )

    # Inputs (per-core data)
    embp = nc.dram_tensor("embp", [128, tot * 129], bf16, kind="ExternalInput").ap()
    embTp = nc.dram_tensor("embTp", [128, tot * 128], f16, kind="ExternalInput").ap()
    segrel = nc.dram_tensor("segrel", [128, tot], f32, kind="ExternalInput").ap()
    embLT = nc.dram_tensor("embLT", [128, gc], f16, kind="ExternalInput").ap()
    # Constants (identical across cores)
    arm = nc.dram_tensor("arm", [128, 128], f16, kind="ExternalInput").ap()
    ucol = nc.dram_tensor("ucol", [128, 1], f32, kind="ExternalInput").ap()
    wkt = nc.dram_tensor("wkt", [128, 128], bf16, kind="ExternalInput").ap()
    iota = nc.dram_tensor("iota", [128, 128], bf16, kind="ExternalInput").ap()
    iotab = nc.dram_tensor("iotab", [128, Cmax * 128], bf16, kind="ExternalInput").ap()
    # Outputs
    outT = nc.dram_tensor("outT", [128, gc], bf16, kind="ExternalOutput").ap()
    dens = nc.dram_tensor("dens", [128, nblk], f32, kind="ExternalOutput").ap()

    # logits-psum chunking (tiles per chunk); phase A chunking
    CH = 6
    ACH = 512
    nach = (gc + ACH - 1) // ACH

    with tile.TileContext(nc) as tc:
        with (
            tc.tile_pool(name="cpool", bufs=1) as cpool,
            tc.tile_pool(name="qpsum", bufs=2, space="PSUM") as qpsum,   # shared: phase A + logits chunks (2 banks each)
            tc.tile_pool(name="bemb", bufs=6) as bemb,
            tc.tile_pool(name="bembt", bufs=6) as bembt,
            tc.tile_pool(name="bmeta", bufs=5) as bmeta,
            tc.tile_pool(name="be", bufs=4) as be,
            tc.tile_pool(name="boh", bufs=4) as boh,
            tc.tile_pool(name="bme", bufs=4) as bme,
            tc.tile_pool(name="bps", bufs=2, space="PSUM") as bps,       # 2 banks
            tc.tile_pool(name="bpo", bufs=2, space="PSUM") as bpo,       # 2 banks
            tc.tile_pool(name="bsb", bufs=2) as bsb,
        ):
            # ---- constants ----
            arm_sb = cpool.tile([128, 128], f16)
            nc.sync.dma_start(out=arm_sb[:], in_=arm)
            ucol_sb = cpool.tile([128, 1], f32)
            nc.sync.dma_start(out=ucol_sb[:], in_=ucol)
            wkt_sb = cpool.tile([128, 128], bf16)
            nc.sync.dma_start(out=wkt_sb[:], in_=wkt)
            iota_sb = cpool.tile([128, 128], bf16)
            nc.sync.dma_start(out=iota_sb[:], in_=iota)
            iotab_sb = cpool.tile([128, Cmax, 128], bf16)
            nc.sync.dma_start(out=iotab_sb[:], in_=iotab)
            embLT_sb = cpool.tile([128, gc], f16)
            nc.sync.dma_start(out=embLT_sb[:], in_=embLT)
            den_all = cpool.tile([128, nblk], f32)
            stsT_all = cpool.tile([128, gc], bf16)   # S_T strips, transposed at the end
            sts_all = cpool.tile([128, nblk, 128], bf16)
            qkT = cpool.tile([128, gc], f16)      # SBUF-resident qk table

            # ---- phase A: qkT[c, g] in 1024-wide chunks ----
            for a in range(nach):
                w = min(ACH, gc - a * ACH)
                qp = qpsum.tile([128, ACH], f32, space="PSUM", tag="acc", padded_shape=[128, CH * 128])
                nc.tensor.matmul(
                    qp[:, :w], lhsT=arm_sb[:], rhs=embLT_sb[:, a * ACH : a * ACH + w],
                    start=True, stop=True,
                )
                nc.scalar.activation(
                    qkT[:, a * ACH : a * ACH + w], qp[:, :w],
                    mybir.ActivationFunctionType.Identity, bias=ucol_sb[:],
                )

            # ---- phase B (software-pipelined: stage1(b) then stage2(b-1)) ----
            offs = [0]
            for cb in C:
                offs.append(offs[-1] + cb)
            state = {}
            proj = {}   # pending blocks for the batched projection

            def stage1(b):
                cb = C[b]
                off = offs[b]
                embt = bemb.tile([128, Cmax * 129], bf16, name=f"embt{b}", tag="embt")
                nc.sync.dma_start(
                    out=embt[:, : cb * 129], in_=embp[:, off * 129 : (off + cb) * 129]
                )
                embT = bembt.tile([128, Cmax, 128], f16, name=f"embT{b}", tag="embT")
                nc.scalar.dma_start(
                    out=embT[:, :cb, :], in_=embTp[:, off * 128 : (off + cb) * 128]
                )
                srel = bmeta.tile([128, Cmax], f32, name=f"srel{b}", tag="srel")
                nc.sync.dma_start(out=srel[:, :cb], in_=segrel[:, off : off + cb])

                # logits in CH-tile chunks; exp chases each chunk
                ebig = be.tile([128, Cmax, 128], bf16, name=f"ebig{b}", tag="ebig")
                for c0 in range(0, cb, CH):
                    cw = min(CH, cb - c0)
                    psumL = qpsum.tile([128, CH * 128], f32, space="PSUM", tag="acc")
                    for t in range(cw):
                        nc.tensor.matmul(
                            psumL[:, ts(t, 128)], lhsT=embT[:, c0 + t, :],
                            rhs=qkT[:, ts(b, 128)], start=True, stop=True,
                        )
                    nc.scalar.activation(
                        ebig[:, c0 : c0 + cw, :], psumL[:, : cw * 128],
                        mybir.ActivationFunctionType.Exp,
                    )
                state[b] = (embt, srel, ebig)

            XB = 32   # blocks per batched SBUF-xbar transpose

            def stage2(b):
                cb = C[b]
                embt, srel, ebig = state.pop(b)
                # one-hot mask, then one batched multiply
                ohb = boh.tile([128, Cmax, 128], bf16, name=f"ohb{b}", tag="ohb")
                nc.vector.tensor_tensor(
                    out=ohb[:, :cb, :],
                    in0=iotab_sb[:, :cb, :],
                    in1=srel[:, :cb].unsqueeze(2).broadcast_to([128, cb, 128]),
                    op=mybir.AluOpType.is_equal,
                )
                meb = bme.tile([128, Cmax, 128], bf16, name=f"meb{b}", tag="meb")
                nc.vector.tensor_tensor(
                    out=meb[:, :cb, :], in0=ohb[:, :cb, :], in1=ebig[:, :cb, :],
                    op=mybir.AluOpType.mult,
                )

                # scatter + denominator in one accumulating matmul chain:
                # spT[g, 0:128] = sum_i me[i,g] * emb[i,c];  spT[g, 128] = den[g]
                spT = bps.tile([128, 129], f32, space="PSUM", name=f"spT{b}", tag="spT")
                for t in range(cb):
                    nc.tensor.matmul(
                        spT[:], lhsT=meb[:, t, :], rhs=embt[:, t * 129 : (t + 1) * 129],
                        start=(t == 0), stop=(t == cb - 1),
                    )
                nc.vector.tensor_copy(den_all[:, b : b + 1], spT[:, 128:129])
                nc.scalar.activation(
                    stsT_all[:, ts(b, 128)], spT[:, :128],
                    mybir.ActivationFunctionType.Copy,
                )
                # batched un-transpose every XB blocks (few xbar-mode toggles),
                # then immediately project those blocks (one Wk load per 4)
                if (b + 1) % XB == 0 or b == nblk - 1:
                    b0 = (b // XB) * XB
                    nbx = b - b0 + 1
                    nc.sync.dma_start_transpose(
                        out=sts_all[:, b0 : b0 + nbx, :],
                        in_=stsT_all[:, b0 * 128 : (b0 + nbx) * 128],
                    )
                    for p0 in range(b0, b0 + nbx, 4):
                        nb = min(4, b0 + nbx - p0)
                        otp4 = bpo.tile([128, 512], f32, space="PSUM", name=f"otp{p0}", tag="otp")
                        nc.tensor.matmul(
                            otp4[:, : nb * 128], lhsT=wkt_sb[:],
                            rhs=sts_all[:, p0 : p0 + nb, :], start=True, stop=True,
                        )
                        ots4 = bsb.tile([128, 512], bf16, name=f"ots{p0}", tag="ots4")
                        nc.scalar.activation(
                            ots4[:, : nb * 128], otp4[:, : nb * 128],
                            mybir.ActivationFunctionType.Copy,
                        )
                        nc.scalar.dma_start(
                            out=outT[:, p0 * 128 : (p0 + nb) * 128], in_=ots4[:, : nb * 128]
                        )

            for b in range(nblk):
                stage1(b)
                if b > 0:
                    stage2(b - 1)
            stage2(nblk - 1)

            nc.sync.dma_start(out=dens, in_=den_all[:])

    nc.compile()
    return nc


def _host_prep(embeddings, seg_ids, Wq, bq, Wk, bk):
    """Bin-pack groups, build per-core arrays + constants, and the output map.

    Returns (C profile, in_maps, perm) where perm[core, slot] = global group id
    (or -1) for slot = b*128 + j.
    """
    emb = np.ascontiguousarray(embeddings, dtype=np.float32)
    seg = np.ascontiguousarray(seg_ids, dtype=np.int64)

    counts = np.bincount(seg, minlength=G)
    cum = np.concatenate([[0], np.cumsum(counts)])   # group g elements: cum[g]:cum[g+1]
    last_idx = np.cumsum(counts) - 1

    ARm = (Wq.T @ Wk).astype(np.float32)
    uvec = (bq @ Wk).astype(np.float32)

    emb_bf = emb.astype(BF16)

    # ---- bin-pack each core's groups into NBLK blocks (<=128 groups each) ----
    # snake deal by descending size, then sort blocks by load desc
    core_blocks = []       # [core][b] -> (group_ids array, load)
    for c in range(NCORES):
        g0 = c * GPC
        gids = np.arange(g0, g0 + GPC)
        sizes = counts[gids]
        order = np.argsort(-sizes, kind="stable")
        sg = gids[order]
        blocks = [[] for _ in range(NBLK)]
        loads = np.zeros(NBLK, dtype=np.int64)
        # snake deal
        pos = 0
        fwd = True
        for k in range(len(sg)):
            idx = pos if fwd else NBLK - 1 - pos
            blocks[idx].append(sg[k])
            loads[idx] += counts[sg[k]]
            pos += 1
            if pos == NBLK:
                pos = 0
                fwd = not fwd
        bo = np.argsort(-loads, kind="stable")
        core_blocks.append([(np.array(blocks[i], dtype=np.int64), int(loads[i])) for i in bo])

    # per-block tile profile shared across cores
    C = []
    for b in range(NBLK):
        mx = max(core_blocks[c][b][1] for c in range(NCORES))
        C.append(max(1, (mx + 127) // 128))
    tot = sum(C)

    iota = np.tile(np.arange(128, dtype=np.float32), (128, 1)).astype(BF16)
    Cmax = max(C)
    consts = dict(
        arm=ARm.astype(FP16),
        ucol=uvec.reshape(128, 1).astype(np.float32),
        wkt=np.ascontiguousarray(Wk.T.astype(np.float32)).astype(BF16),
        iota=iota,
        iotab=np.ascontiguousarray(np.tile(iota, (1, Cmax))),
    )

    in_maps = []
    perm = np.full((NCORES, GC), -1, dtype=np.int64)
    offs = np.concatenate([[0], np.cumsum(C)]).astype(np.int64)
    for c in range(NCORES):
        # packed group order (block-major), with per-group block id and column
        gorder = np.concatenate([core_blocks[c][b][0] for b in range(NBLK)])
        gblk = np.concatenate(
            [np.full(len(core_blocks[c][b][0]), b, dtype=np.int64) for b in range(NBLK)]
        )
        gj = np.concatenate(
            [np.arange(len(core_blocks[c][b][0]), dtype=np.int64) for b in range(NBLK)]
        )
        lens = counts[gorder]
        ne = int(lens.sum())
        # element global indices = concatenated ranges cum[g]:cum[g+1]
        lens_cum = np.concatenate([[0], np.cumsum(lens)[:-1]]).astype(np.int64)
        within = np.arange(ne, dtype=np.int64) - np.repeat(lens_cum, lens)
        eidx = np.repeat(cum[gorder], lens) + within
        eblk = np.repeat(gblk, lens)
        ej = np.repeat(gj, lens)
        # position within block (elements are in block-major order)
        blk_sizes = np.bincount(eblk, minlength=NBLK).astype(np.int64)
        blk_start = np.concatenate([[0], np.cumsum(blk_sizes)[:-1]]).astype(np.int64)
        pos = np.arange(ne, dtype=np.int64) - np.repeat(blk_start, blk_sizes)
        t = pos // 128
        i = pos % 128
        Tg = offs[eblk] + t          # global tile index in [0, tot)

        embp3 = np.zeros((128, tot, 129), dtype=BF16)
        embp3[:, :, 128] = BF16(1.0)
        embp3[i, Tg, :128] = emb_bf[eidx]
        embT3 = np.zeros((128, tot, 128), dtype=FP16)
        embT3[:, Tg, i] = emb[eidx].T.astype(FP16)
        segrel = np.full((128, tot), -1.0, dtype=np.float32)
        segrel[i, Tg] = ej.astype(np.float32)
        embLT = np.zeros((128, GC), dtype=FP16)
        embLT[:, gblk * 128 + gj] = emb[last_idx[gorder]].T.astype(FP16)
        perm[c, gblk * 128 + gj] = gorder

        m = dict(
            embp=np.ascontiguousarray(embp3.reshape(128, tot * 129)),
            embTp=np.ascontiguousarray(embT3.reshape(128, tot * 128)),
            segrel=np.ascontiguousarray(segrel),
            embLT=np.ascontiguousarray(embLT),
        )
        m.update(consts)
        in_maps.append(m)
    return C, in_maps, perm


def kernel(embeddings, seg_ids, Wq, bq, Wk, bk):
    global LAST_EXEC_NS, LAST_RESULTS
    Wq = np.asarray(Wq, dtype=np.float32)
    bq = np.asarray(bq, dtype=np.float32)
    Wk = np.asarray(Wk, dtype=np.float32)
    bk = np.asarray(bk, dtype=np.float32)
    embeddings = np.asarray(embeddings)
    seg_ids = np.asarray(seg_ids)

    C, in_maps, perm = _host_prep(embeddings, seg_ids, Wq, bq, Wk, bk)

    key = tuple(C)
    if key not in _cache:
        _cache[key] = _build_program(C)
    nc = _cache[key]

    trace = bool(int(os.environ.get("BASS_KERNEL_TRACE", "0")))
    res = run_bass_kernel_spmd(nc, in_maps, core_ids=list(range(NCORES)), trace=trace)
    LAST_RESULTS = res
    LAST_EXEC_NS = res.exec_time_ns

    out = np.empty((G, D), dtype=np.float32)
    for c in range(NCORES):
        oT = res.results[c]["outT"].astype(np.float32)     # [128, GC]
        dn = res.results[c]["dens"].T.reshape(-1)          # [128, NBLK] -> slot b*128+j
        valid = perm[c] >= 0
        out[perm[c, valid]] = oT[:, valid].T / dn[valid, None] + bk
    return out


# revision 41
# speedup vs baseline: 1.3046x; 1.3046x over previous
"""Trainium2 Bass kernel for nn_Attentional_Aggregation (segment softmax attention).

Math (reference):
    keys_i = emb_i @ Wk.T + bk
    q_g    = emb[last(g)] @ Wq.T + bq
    logit_i = <q_{g(i)}, keys_i>
    w = segment_softmax(logit)
    out_g = sum_{i in g} w_i * keys_i

Reformulation:
    logit_i = <qk_{g(i)}, emb_i>,  qk_g = embL_g @ (Wq.T Wk) + bq Wk
    out_g = (sum e_i emb_i / sum e_i) @ Wk.T + bk   (device: Wk @ S and denom;
                                                     host: divide + bias)

Device strategy (per core, fully static SPMD program):
  phase A: qkT[c, g] = ARm.T @ embLT (+ u bias via ACT), SBUF-resident, 512-chunks.
  phase B, software-pipelined (stage1(b) || stage2(b-1)) over 98 blocks
  (128 groups, C_b element tiles of 128 each):
    stage1: plain DMAs of natural emb [i, t*129+c] bf16 (col 128 = ones) and
      host-pretransposed embT [c, t*128+i] f16; per tile logits MM
      (lhsT=embT_t, rhs=qkT_blk) into [128, CH*128] PSUM chunks; one EXP per
      chunk -> E bf16 (ACT).
    stage2: one-hot ohb = is_eq(iotab, srel bcast) + one batched multiply
      me = ohb*E (DVE); flip-scatter: spT[g, 0:129] += me_t.T @ [emb_t | 1]
      accumulates BOTH the numerator S_T[g,c] and the denominator (col 128)
      in a single MM chain per tile; den col + S_T strip copied out (DVE/ACT).
  Every 32 blocks: ONE batched SBUF->SBUF xbar transpose turns the S_T strips
  into S [c, b, g] (few xbar-mode toggles - per-block transposes serialize the
  DMA pipeline), then projections otp = Wk @ S batched 4 blocks per weight
  load; bf16 outT. Host: divide by den, add bk, un-permute groups.

Sharding: 12500 groups per core; groups bin-packed (snake by size) into 98
blocks of <=128 groups to equalize per-block element counts; per-block tile
counts C_b shared across cores (max profile). Host un-permutes the output.
"""

import os
import numpy as np
import ml_dtypes

import concourse.bacc as bacc
import concourse.bass as bass
import concourse.mybir as mybir
import concourse.tile as tile
from concourse.bass_utils import run_bass_kernel_spmd

BF16 = ml_dtypes.bfloat16
FP16 = np.float16

N = 1_000_000
G = 100_000
D = 128
NCORES = 8
NBLK = 98
GPC = G // NCORES          # groups per core (12500)
GC = NBLK * 128            # group slots per core (12544)

# Exposed for test harness
LAST_EXEC_NS = None
LAST_RESULTS = None

_cache = {}


def _build_program(C, ncores=NCORES, enable_asserts=False):
    """Build the SPMD Bass program. C = per-block tile counts (len NBLK)."""
    C = list(C)
    nblk = len(C)
    Cmax = max(C)
    assert Cmax <= 12, f"PSUM budget assumes Cmax<=12, got {Cmax}"
    tot = sum(C)
    f32 = mybir.dt.float32
    bf16 = mybir.dt.bfloat16
    f16 = mybir.dt.float16
    ts = bass.ts
    gc = nblk * 128

    nc = bacc.Bacc(
        "TRN2",
        target_bir_lowering=False,
        debug=False,
        enable_asserts=enable_asserts,
        num_devices=ncores,
    )

    # Inputs (per-core data)
    embp = nc.dram_tensor("embp", [128, tot * 129], bf16, kind="ExternalInput").ap()
    embTp = nc.dram_tensor("embTp", [128, tot * 128], f16, kind="ExternalInput").ap()
    segrel = nc.dram_tensor("segrel", [128, tot], f32, kind="ExternalInput").ap()
    embLT = nc.dram_tensor("embLT", [128, gc], f16, kind="ExternalInput").ap()
    # Constants (identical across cores)
    arm = nc.dram_tensor("arm", [128, 128], f16, kind="ExternalInput").ap()
    ucol = nc.dram_tensor("ucol", [128, 1], f32, kind="ExternalInput").ap()
    wkt = nc.dram_tensor("wkt", [128, 128], bf16, kind="ExternalInput").ap()
    iota = nc.dram_tensor("iota", [128, 128], bf16, kind="ExternalInput").ap()
    iotab = nc.dram_tensor("iotab", [128, Cmax * 128], bf16, kind="ExternalInput").ap()
    # Outputs
    outT = nc.dram_tensor("outT", [128, gc], bf16, kind="ExternalOutput").ap()
    dens = nc.dram_tensor("dens", [128, nblk], f32, kind="ExternalOutput").ap()

    # logits-psum chunking (tiles per chunk); phase A chunking
    CH = 6
    ACH = 512
    nach = (gc + ACH - 1) // ACH

    with tile.TileContext(nc) as tc:
        with (
            tc.tile_pool(name="cpool", bufs=1) as cpool,
            tc.tile_pool(name="qpsum", bufs=2, space="PSUM") as qpsum,   # shared: phase A + logits chunks (2 banks each)
            tc.tile_pool(name="bemb", bufs=6) as bemb,
            tc.tile_pool(name="bembt", bufs=6) as bembt,
            tc.tile_pool(name="bmeta", bufs=5) as bmeta,
            tc.tile_pool(name="be", bufs=5) as be,
            tc.tile_pool(name="boh", bufs=5) as boh,
            tc.tile_pool(name="bme", bufs=5) as bme,
            tc.tile_pool(name="bps", bufs=2, space="PSUM") as bps,       # 2 banks
            tc.tile_pool(name="bpo", bufs=2, space="PSUM") as bpo,       # 2 banks
            tc.tile_pool(name="bsb", bufs=3) as bsb,
        ):
            # ---- constants ----
            arm_sb = cpool.tile([128, 128], f16)
            nc.sync.dma_start(out=arm_sb[:], in_=arm)
            ucol_sb = cpool.tile([128, 1], f32)
            nc.sync.dma_start(out=ucol_sb[:], in_=ucol)
            wkt_sb = cpool.tile([128, 128], bf16)
            nc.sync.dma_start(out=wkt_sb[:], in_=wkt)
            iota_sb = cpool.tile([128, 128], bf16)
            nc.sync.dma_start(out=iota_sb[:], in_=iota)
            iotab_sb = cpool.tile([128, Cmax, 128], bf16)
            nc.sync.dma_start(out=iotab_sb[:], in_=iotab)
            embLT_sb = cpool.tile([128, gc], f16)
            nc.sync.dma_start(out=embLT_sb[:], in_=embLT)
            den_all = cpool.tile([128, nblk], f32)
            stsT_all = cpool.tile([128, gc], bf16)   # S_T strips, transposed at the end
            sts_all = cpool.tile([128, nblk, 128], bf16)
            qkT = cpool.tile([128, gc], f16)      # SBUF-resident qk table

            # ---- phase A: qkT[c, g] in 1024-wide chunks ----
            for a in range(nach):
                w = min(ACH, gc - a * ACH)
                qp = qpsum.tile([128, ACH], f32, space="PSUM", tag="acc", padded_shape=[128, CH * 128])
                nc.tensor.matmul(
                    qp[:, :w], lhsT=arm_sb[:], rhs=embLT_sb[:, a * ACH : a * ACH + w],
                    start=True, stop=True,
                )
                nc.scalar.activation(
                    qkT[:, a * ACH : a * ACH + w], qp[:, :w],
                    mybir.ActivationFunctionType.Identity, bias=ucol_sb[:],
                )

            # ---- phase B (software-pipelined: stage1(b) then stage2(b-1)) ----
            offs = [0]
            for cb in C:
                offs.append(offs[-1] + cb)
            state = {}
            proj = {}   # pending blocks for the batched projection

            def stage1(b):
                cb = C[b]
                off = offs[b]
                embt = bemb.tile([128, Cmax * 129], bf16, name=f"embt{b}", tag="embt")
                nc.sync.dma_start(
                    out=embt[:, : cb * 129], in_=embp[:, off * 129 : (off + cb) * 129]
                )
                embT = bembt.tile([128, Cmax, 128], f16, name=f"embT{b}", tag="embT")
                nc.sync.dma_start(
                    out=embT[:, :cb, :], in_=embTp[:, off * 128 : (off + cb) * 128]
                )
                srel = bmeta.tile([128, Cmax], f32, name=f"srel{b}", tag="srel")
                nc.sync.dma_start(out=srel[:, :cb], in_=segrel[:, off : off + cb])

                # logits in CH-tile chunks; exp chases each chunk
                ebig = be.tile([128, Cmax, 128], bf16, name=f"ebig{b}", tag="ebig")
                for c0 in range(0, cb, CH):
                    cw = min(CH, cb - c0)
                    psumL = qpsum.tile([128, CH * 128], f32, space="PSUM", tag="acc")
                    for t in range(cw):
                        nc.tensor.matmul(
                            psumL[:, ts(t, 128)], lhsT=embT[:, c0 + t, :],
                            rhs=qkT[:, ts(b, 128)], start=True, stop=True,
                        )
                    nc.scalar.activation(
                        ebig[:, c0 : c0 + cw, :], psumL[:, : cw * 128],
                        mybir.ActivationFunctionType.Exp,
                    )
                state[b] = (embt, srel, ebig)

            XB = 32   # blocks per batched SBUF-xbar transpose

            def stage2(b):
                cb = C[b]
                embt, srel, ebig = state.pop(b)
                # one-hot mask, then one batched multiply
                ohb = boh.tile([128, Cmax, 128], bf16, name=f"ohb{b}", tag="ohb")
                nc.vector.tensor_tensor(
                    out=ohb[:, :cb, :],
                    in0=iotab_sb[:, :cb, :],
                    in1=srel[:, :cb].unsqueeze(2).broadcast_to([128, cb, 128]),
                    op=mybir.AluOpType.is_equal,
                )
                meb = bme.tile([128, Cmax, 128], bf16, name=f"meb{b}", tag="meb")
                nc.vector.tensor_tensor(
                    out=meb[:, :cb, :], in0=ohb[:, :cb, :], in1=ebig[:, :cb, :],
                    op=mybir.AluOpType.mult,
                )

                # scatter + denominator in one accumulating matmul chain:
                # spT[g, 0:128] = sum_i me[i,g] * emb[i,c];  spT[g, 128] = den[g]
                spT = bps.tile([128, 129], f32, space="PSUM", name=f"spT{b}", tag="spT")
                for t in range(cb):
                    nc.tensor.matmul(
                        spT[:], lhsT=meb[:, t, :], rhs=embt[:, t * 129 : (t + 1) * 129],
                        start=(t == 0), stop=(t == cb - 1),
                    )
                nc.vector.tensor_copy(den_all[:, b : b + 1], spT[:, 128:129])
                nc.scalar.activation(
                    stsT_all[:, ts(b, 128)], spT[:, :128],
                    mybir.ActivationFunctionType.Copy,
                )
                # batched un-transpose every XB blocks (few xbar-mode toggles),
                # then immediately project those blocks (one Wk load per 4)
                if (b + 1) % XB == 0 or b == nblk - 1:
                    b0 = (b // XB) * XB
                    nbx = b - b0 + 1
                    nc.sync.dma_start_transpose(
                        out=sts_all[:, b0 : b0 + nbx, :],
                        in_=stsT_all[:, b0 * 128 : (b0 + nbx) * 128],
                    )
                    for p0 in range(b0, b0 + nbx, 4):
                        nb = min(4, b0 + nbx - p0)
                        otp4 = bpo.tile([128, 512], f32, space="PSUM", name=f"otp{p0}", tag="otp")
                        nc.tensor.matmul(
                            otp4[:, : nb * 128], lhsT=wkt_sb[:],
                            rhs=sts_all[:, p0 : p0 + nb, :], start=True, stop=True,
                        )
                        ots4 = bsb.tile([128, 512], bf16, name=f"ots{p0}", tag="ots4")
                        nc.scalar.activation(
                            ots4[:, : nb * 128], otp4[:, : nb * 128],
                            mybir.ActivationFunctionType.Copy,
                        )
                        nc.sync.dma_start(
                            out=outT[:, p0 * 128 : (p0 + nb) * 128], in_=ots4[:, : nb * 128]
                        )

            for b in range(nblk):
                stage1(b)
                if b > 0:
                    stage2(b - 1)
            stage2(nblk - 1)

            nc.sync.dma_start(out=dens, in_=den_all[:])

    nc.compile()
    return nc


def _host_prep(embeddings, seg_ids, Wq, bq, Wk, bk):
    """Bin-pack groups, build per-core arrays + constants, and the output map.

    Returns (C profile, in_maps, perm) where perm[core, slot] = global group id
    (or -1) for slot = b*128 + j.
    """
    emb = np.ascontiguousarray(embeddings, dtype=np.float32)
    seg = np.ascontiguousarray(seg_ids, dtype=np.int64)

    counts = np.bincount(seg, minlength=G)
    cum = np.concatenate([[0], np.cumsum(counts)])   # group g elements: cum[g]:cum[g+1]
    last_idx = np.cumsum(counts) - 1

    ARm = (Wq.T @ Wk).astype(np.float32)
    uvec = (bq @ Wk).astype(np.float32)

    emb_bf = emb.astype(BF16)

    # ---- bin-pack each core's groups into NBLK blocks (<=128 groups each) ----
    # snake deal by descending size, then sort blocks by load desc
    core_blocks = []       # [core][b] -> (group_ids array, load)
    for c in range(NCORES):
        g0 = c * GPC
        gids = np.arange(g0, g0 + GPC)
        sizes = counts[gids]
        order = np.argsort(-sizes, kind="stable")
        sg = gids[order]
        blocks = [[] for _ in range(NBLK)]
        loads = np.zeros(NBLK, dtype=np.int64)
        # snake deal
        pos = 0
        fwd = True
        for k in range(len(sg)):
            idx = pos if fwd else NBLK - 1 - pos
            blocks[idx].append(sg[k])
            loads[idx] += counts[sg[k]]
            pos += 1
            if pos == NBLK:
                pos = 0
                fwd = not fwd
        bo = np.argsort(-loads, kind="stable")
        core_blocks.append([(np.array(blocks[i], dtype=np.int64), int(loads[i])) for i in bo])

    # per-block tile profile shared across cores
    C = []
    for b in range(NBLK):
        mx = max(core_blocks[c][b][1] for c in range(NCORES))
        C.append(max(1, (mx + 127) // 128))
    tot = sum(C)

    iota = np.tile(np.arange(128, dtype=np.float32), (128, 1)).astype(BF16)
    Cmax = max(C)
    consts = dict(
        arm=ARm.astype(FP16),
        ucol=uvec.reshape(128, 1).astype(np.float32),
        wkt=np.ascontiguousarray(Wk.T.astype(np.float32)).astype(BF16),
        iota=iota,
        iotab=np.ascontiguousarray(np.tile(iota, (1, Cmax))),
    )

    in_maps = []
    perm = np.full((NCORES, GC), -1, dtype=np.int64)
    offs = np.concatenate([[0], np.cumsum(C)]).astype(np.int64)
    for c in range(NCORES):
        # packed group order (block-major), with per-group block id and column
        gorder = np.concatenate([core_blocks[c][b][0] for b in range(NBLK)])
        gblk = np.concatenate(
            [np.full(len(core_blocks[c][b][0]), b, dtype=np.int64) for b in range(NBLK)]
        )
        gj = np.concatenate(
            [np.arange(len(core_blocks[c][b][0]), dtype=np.int64) for b in range(NBLK)]
        )
        lens = counts[gorder]
        ne = int(lens.sum())
        # element global indices = concatenated ranges cum[g]:cum[g+1]
        lens_cum = np.concatenate([[0], np.cumsum(lens)[:-1]]).astype(np.int64)
        within = np.arange(ne, dtype=np.int64) - np.repeat(lens_cum, lens)
        eidx = np.repeat(cum[gorder], lens) + within
        eblk = np.repeat(gblk, lens)
        ej = np.repeat(gj, lens)
        # position within block (elements are in block-major order)
        blk_sizes = np.bincount(eblk, minlength=NBLK).astype(np.int64)
        blk_start = np.concatenate([[0], np.cumsum(blk_sizes)[:-1]]).astype(np.int64)
        pos = np.arange(ne, dtype=np.int64) - np.repeat(blk_start, blk_sizes)
        t = pos // 128
        i = pos % 128
        Tg = offs[eblk] + t          # global tile index in [0, tot)

        embp3 = np.zeros((128, tot, 129), dtype=BF16)
        embp3[:, :, 128] = BF16(1.0)
        embp3[i, Tg, :128] = emb_bf[eidx]
        embT3 = np.zeros((128, tot, 128), dtype=FP16)
        embT3[:, Tg, i] = emb[eidx].T.astype(FP16)
        segrel = np.full((128, tot), -1.0, dtype=np.float32)
        segrel[i, Tg] = ej.astype(np.float32)
        embLT = np.zeros((128, GC), dtype=FP16)
        embLT[:, gblk * 128 + gj] = emb[last_idx[gorder]].T.astype(FP16)
        perm[c, gblk * 128 + gj] = gorder

        m = dict(
            embp=np.ascontiguousarray(embp3.reshape(128, tot * 129)),
            embTp=np.ascontiguousarray(embT3.reshape(128, tot * 128)),
            segrel=np.ascontiguousarray(segrel),
            embLT=np.ascontiguousarray(embLT),
        )
        m.update(consts)
        in_maps.append(m)
    return C, in_maps, perm


def kernel(embeddings, seg_ids, Wq, bq, Wk, bk):
    global LAST_EXEC_NS, LAST_RESULTS
    Wq = np.asarray(Wq, dtype=np.float32)
    bq = np.asarray(bq, dtype=np.float32)
    Wk = np.asarray(Wk, dtype=np.float32)
    bk = np.asarray(bk, dtype=np.float32)
    embeddings = np.asarray(embeddings)
    seg_ids = np.asarray(seg_ids)

    C, in_maps, perm = _host_prep(embeddings, seg_ids, Wq, bq, Wk, bk)

    key = tuple(C)
    if key not in _cache:
        _cache[key] = _build_program(C)
    nc = _cache[key]

    trace = bool(int(os.environ.get("BASS_KERNEL_TRACE", "0")))
    res = run_bass_kernel_spmd(nc, in_maps, core_ids=list(range(NCORES)), trace=trace)
    LAST_RESULTS = res
    LAST_EXEC_NS = res.exec_time_ns

    out = np.empty((G, D), dtype=np.float32)
    for c in range(NCORES):
        oT = res.results[c]["outT"].astype(np.float32)     # [128, GC]
        dn = res.results[c]["dens"].T.reshape(-1)          # [128, NBLK] -> slot b*128+j
        valid = perm[c] >= 0
        out[perm[c, valid]] = oT[:, valid].T / dn[valid, None] + bk
    return out


# revision 43
# speedup vs baseline: 1.3516x; 1.0360x over previous
"""Trainium2 Bass kernel for nn_Attentional_Aggregation (segment softmax attention).

Math (reference):
    keys_i = emb_i @ Wk.T + bk
    q_g    = emb[last(g)] @ Wq.T + bq
    logit_i = <q_{g(i)}, keys_i>
    w = segment_softmax(logit)
    out_g = sum_{i in g} w_i * keys_i

Reformulation:
    logit_i = <qk_{g(i)}, emb_i>,  qk_g = embL_g @ (Wq.T Wk) + bq Wk
    out_g = (sum e_i emb_i / sum e_i) @ Wk.T + bk   (device: Wk @ S and denom;
                                                     host: divide + bias)

Device strategy (per core, fully static SPMD program):
  phase A: qkT[c, g] = ARm.T @ embLT (+ u bias via ACT), SBUF-resident, 512-chunks.
  phase B, software-pipelined (stage1(b) || stage2(b-1)) over 98 blocks
  (128 groups, C_b element tiles of 128 each):
    stage1: plain DMAs of natural emb [i, t*129+c] bf16 (col 128 = ones) and
      host-pretransposed embT [c, t*128+i] f16; per tile logits MM
      (lhsT=embT_t, rhs=qkT_blk) into [128, CH*128] PSUM chunks; one EXP per
      chunk -> E bf16 (ACT).
    stage2: one-hot ohb = is_eq(iotab, srel bcast) + one batched multiply
      me = ohb*E (DVE); flip-scatter: spT[g, 0:129] += me_t.T @ [emb_t | 1]
      accumulates BOTH the numerator S_T[g,c] and the denominator (col 128)
      in a single MM chain per tile; den col + S_T strip copied out (DVE/ACT).
  Every 32 blocks: ONE batched SBUF->SBUF xbar transpose turns the S_T strips
  into S [c, b, g] (few xbar-mode toggles - per-block transposes serialize the
  DMA pipeline), then projections otp = Wk @ S batched 4 blocks per weight
  load; bf16 outT. Host: divide by den, add bk, un-permute groups.

Sharding: 12500 groups per core; groups bin-packed (snake by size) into 98
blocks of <=128 groups to equalize per-block element counts; per-block tile
counts C_b shared across cores (max profile). Host un-permutes the output.
"""

import os
import numpy as np
import ml_dtypes

import concourse.bacc as bacc
import concourse.bass as bass
import concourse.mybir as mybir
import concourse.tile as tile
from concourse.bass_utils import run_bass_kernel_spmd

BF16 = ml_dtypes.bfloat16
FP16 = np.float16

N = 1_000_000
G = 100_000
D = 128
NCORES = 8
NBLK = 98
GPC = G // NCORES          # groups per core (12500)
GC = NBLK * 128            # group slots per core (12544)

# Exposed for test harness
LAST_EXEC_NS = None
LAST_RESULTS = None

_cache = {}


def _build_program(C, ncores=NCORES, enable_asserts=False):
    """Build the SPMD Bass program. C = per-block tile counts (len NBLK)."""
    C = list(C)
    nblk = len(C)
    Cmax = max(C)
    assert Cmax <= 12, f"PSUM budget assumes Cmax<=12, got {Cmax}"
    tot = sum(C)
    f32 = mybir.dt.float32
    bf16 = mybir.dt.bfloat16
    f16 = mybir.dt.float16
    ts = bass.ts
    gc = nblk * 128

    nc = bacc.Bacc(
        "TRN2",
        target_bir_lowering=False,
        debug=False,
        enable_asserts=enable_asserts,
        num_devices=ncores,
    )

    # Inputs (per-core data)
    embp = nc.dram_tensor("embp", [128, tot * 129], bf16, kind="ExternalInput").ap()
    embTp = nc.dram_tensor("embTp", [128, tot * 128], f16, kind="ExternalInput").ap()
    segrel = nc.dram_tensor("segrel", [128, tot], f32, kind="ExternalInput").ap()
    embLT = nc.dram_tensor("embLT", [128, gc], f16, kind="ExternalInput").ap()
    # Constants (identical across cores)
    arm = nc.dram_tensor("arm", [128, 128], f16, kind="ExternalInput").ap()
    ucol = nc.dram_tensor("ucol", [128, 1], f32, kind="ExternalInput").ap()
    wkt = nc.dram_tensor("wkt", [128, 128], bf16, kind="ExternalInput").ap()
    iota = nc.dram_tensor("iota", [128, 128], bf16, kind="ExternalInput").ap()
    iotab = nc.dram_tensor("iotab", [128, Cmax * 128], bf16, kind="ExternalInput").ap()
    # Outputs
    outT = nc.dram_tensor("outT", [128, gc], bf16, kind="ExternalOutput").ap()
    dens = nc.dram_tensor("dens", [128, nblk], f32, kind="ExternalOutput").ap()

    # logits-psum chunking (tiles per chunk); phase A chunking
    CH = 6
    ACH = 512
    nach = (gc + ACH - 1) // ACH

    with tile.TileContext(nc) as tc:
        with (
            tc.tile_pool(name="cpool", bufs=1) as cpool,
            tc.tile_pool(name="qpsum", bufs=2, space="PSUM") as qpsum,   # shared: phase A + logits chunks (2 banks each)
            tc.tile_pool(name="bemb", bufs=5) as bemb,
            tc.tile_pool(name="bembt", bufs=5) as bembt,
            tc.tile_pool(name="bmeta", bufs=5) as bmeta,
            tc.tile_pool(name="be", bufs=4) as be,
            tc.tile_pool(name="boh", bufs=4) as boh,
            tc.tile_pool(name="bme", bufs=4) as bme,
            tc.tile_pool(name="bps", bufs=2, space="PSUM") as bps,       # 2 banks
            tc.tile_pool(name="bpo", bufs=2, space="PSUM") as bpo,       # 2 banks
            tc.tile_pool(name="bsb", bufs=2) as bsb,
        ):
            # ---- constants ----
            arm_sb = cpool.tile([128, 128], f16)
            nc.sync.dma_start(out=arm_sb[:], in_=arm)
            ucol_sb = cpool.tile([128, 1], f32)
            nc.sync.dma_start(out=ucol_sb[:], in_=ucol)
            wkt_sb = cpool.tile([128, 128], bf16)
            nc.sync.dma_start(out=wkt_sb[:], in_=wkt)
            iota_sb = cpool.tile([128, 128], bf16)
            nc.sync.dma_start(out=iota_sb[:], in_=iota)
            iotab_sb = cpool.tile([128, Cmax, 128], bf16)
            nc.sync.dma_start(out=iotab_sb[:], in_=iotab)
            embLT_sb = cpool.tile([128, gc], f16)
            nc.sync.dma_start(out=embLT_sb[:], in_=embLT)
            den_all = cpool.tile([128, nblk], f32)
            stsT_all = cpool.tile([128, gc], bf16)   # S_T strips, transposed at the end
            sts_all = cpool.tile([128, nblk, 128], bf16)
            qkT = cpool.tile([128, gc], f16)      # SBUF-resident qk table

            # ---- phase A chunks are interleaved into the block loop: block b
            # only needs qkT chunk b//4, so chunk emission rides ahead of it ----
            def phaseA_chunk(a):
                w = min(ACH, gc - a * ACH)
                qp = qpsum.tile([128, ACH], f32, space="PSUM", tag="acc",
                                padded_shape=[128, CH * 128], name=f"qp{a}")
                nc.tensor.matmul(
                    qp[:, :w], lhsT=arm_sb[:], rhs=embLT_sb[:, a * ACH : a * ACH + w],
                    start=True, stop=True,
                )
                nc.scalar.activation(
                    qkT[:, a * ACH : a * ACH + w], qp[:, :w],
                    mybir.ActivationFunctionType.Identity, bias=ucol_sb[:],
                )

            # ---- phase B (software-pipelined: stage1(b) then stage2(b-1)) ----
            offs = [0]
            for cb in C:
                offs.append(offs[-1] + cb)
            state = {}
            proj = {}   # pending blocks for the batched projection

            def stage1(b):
                cb = C[b]
                off = offs[b]
                embt = bemb.tile([128, Cmax * 129], bf16, name=f"embt{b}", tag="embt")
                nc.sync.dma_start(
                    out=embt[:, : cb * 129], in_=embp[:, off * 129 : (off + cb) * 129]
                )
                embT = bembt.tile([128, Cmax, 128], f16, name=f"embT{b}", tag="embT")
                nc.sync.dma_start(
                    out=embT[:, :cb, :], in_=embTp[:, off * 128 : (off + cb) * 128]
                )
                srel = bmeta.tile([128, Cmax], f32, name=f"srel{b}", tag="srel")
                nc.sync.dma_start(out=srel[:, :cb], in_=segrel[:, off : off + cb])

                # logits in CH-tile chunks; exp chases each chunk
                ebig = be.tile([128, Cmax, 128], bf16, name=f"ebig{b}", tag="ebig")
                for c0 in range(0, cb, CH):
                    cw = min(CH, cb - c0)
                    psumL = qpsum.tile([128, CH * 128], f32, space="PSUM", tag="acc")
                    for t in range(cw):
                        nc.tensor.matmul(
                            psumL[:, ts(t, 128)], lhsT=embT[:, c0 + t, :],
                            rhs=qkT[:, ts(b, 128)], start=True, stop=True,
                        )
                    nc.scalar.activation(
                        ebig[:, c0 : c0 + cw, :], psumL[:, : cw * 128],
                        mybir.ActivationFunctionType.Exp,
                    )
                state[b] = (embt, srel, ebig)

            XB = 32   # blocks per batched SBUF-xbar transpose

            def stage2(b):
                cb = C[b]
                embt, srel, ebig = state.pop(b)
                # one-hot mask, then one batched multiply
                ohb = boh.tile([128, Cmax, 128], bf16, name=f"ohb{b}", tag="ohb")
                nc.vector.tensor_tensor(
                    out=ohb[:, :cb, :],
                    in0=iotab_sb[:, :cb, :],
                    in1=srel[:, :cb].unsqueeze(2).broadcast_to([128, cb, 128]),
                    op=mybir.AluOpType.is_equal,
                )
                meb = bme.tile([128, Cmax, 128], bf16, name=f"meb{b}", tag="meb")
                nc.vector.tensor_tensor(
                    out=meb[:, :cb, :], in0=ohb[:, :cb, :], in1=ebig[:, :cb, :],
                    op=mybir.AluOpType.mult,
                )

                # scatter + denominator in one accumulating matmul chain:
                # spT[g, 0:128] = sum_i me[i,g] * emb[i,c];  spT[g, 128] = den[g]
                spT = bps.tile([128, 129], f32, space="PSUM", name=f"spT{b}", tag="spT")
                for t in range(cb):
                    nc.tensor.matmul(
                        spT[:], lhsT=meb[:, t, :], rhs=embt[:, t * 129 : (t + 1) * 129],
                        start=(t == 0), stop=(t == cb - 1),
                    )
                nc.vector.tensor_copy(den_all[:, b : b + 1], spT[:, 128:129])
                nc.scalar.activation(
                    stsT_all[:, ts(b, 128)], spT[:, :128],
                    mybir.ActivationFunctionType.Copy,
                )
                # batched un-transpose every XB blocks (few xbar-mode toggles),
                # then immediately project those blocks (one Wk load per 4)
                if (b + 1) % XB == 0 or b == nblk - 1:
                    b0 = (b // XB) * XB
                    nbx = b - b0 + 1
                    nc.sync.dma_start_transpose(
                        out=sts_all[:, b0 : b0 + nbx, :],
                        in_=stsT_all[:, b0 * 128 : (b0 + nbx) * 128],
                    )
                    for p0 in range(b0, b0 + nbx, 4):
                        nb = min(4, b0 + nbx - p0)
                        otp4 = bpo.tile([128, 512], f32, space="PSUM", name=f"otp{p0}", tag="otp")
                        nc.tensor.matmul(
                            otp4[:, : nb * 128], lhsT=wkt_sb[:],
                            rhs=sts_all[:, p0 : p0 + nb, :], start=True, stop=True,
                        )
                        ots4 = bsb.tile([128, 512], bf16, name=f"ots{p0}", tag="ots4")
                        nc.scalar.activation(
                            ots4[:, : nb * 128], otp4[:, : nb * 128],
                            mybir.ActivationFunctionType.Copy,
                        )
                        nc.sync.dma_start(
                            out=outT[:, p0 * 128 : (p0 + nb) * 128], in_=ots4[:, : nb * 128]
                        )

            for b in range(nblk):
                if b % 4 == 0 and b // 4 < nach:
                    phaseA_chunk(b // 4)
                stage1(b)
                if b > 0:
                    stage2(b - 1)
            stage2(nblk - 1)

            nc.sync.dma_start(out=dens, in_=den_all[:])

    nc.compile()
    return nc


def _host_prep(embeddings, seg_ids, Wq, bq, Wk, bk):
    """Bin-pack groups, build per-core arrays + constants, and the output map.

    Returns (C profile, in_maps, perm) where perm[core, slot] = global group id
    (or -1) for slot = b*128 + j.
    """
    emb = np.ascontiguousarray(embeddings, dtype=np.float32)
    seg = np.ascontiguousarray(seg_ids, dtype=np.int64)

    counts = np.bincount(seg, minlength=G)
    cum = np.concatenate([[0], np.cumsum(counts)])   # group g elements: cum[g]:cum[g+1]
    last_idx = np.cumsum(counts) - 1

    ARm = (Wq.T @ Wk).astype(np.float32)
    uvec = (bq @ Wk).astype(np.float32)

    emb_bf = emb.astype(BF16)

    # ---- bin-pack each core's groups into NBLK blocks (<=128 groups each) ----
    # snake deal by descending size, then sort blocks by load desc
    core_blocks = []       # [core][b] -> (group_ids array, load)
    for c in range(NCORES):
        g0 = c * GPC
        gids = np.arange(g0, g0 + GPC)
        sizes = counts[gids]
        order = np.argsort(-sizes, kind="stable")
        sg = gids[order]
        blocks = [[] for _ in range(NBLK)]
        loads = np.zeros(NBLK, dtype=np.int64)
        # snake deal
        pos = 0
        fwd = True
        for k in range(len(sg)):
            idx = pos if fwd else NBLK - 1 - pos
            blocks[idx].append(sg[k])
            loads[idx] += counts[sg[k]]
            pos += 1
            if pos == NBLK:
                pos = 0
                fwd = not fwd
        bo = np.argsort(-loads, kind="stable")
        core_blocks.append([(np.array(blocks[i], dtype=np.int64), int(loads[i])) for i in bo])

    # per-block tile profile shared across cores
    C = []
    for b in range(NBLK):
        mx = max(core_blocks[c][b][1] for c in range(NCORES))
        C.append(max(1, (mx + 127) // 128))
    tot = sum(C)

    iota = np.tile(np.arange(128, dtype=np.float32), (128, 1)).astype(BF16)
    Cmax = max(C)
    consts = dict(
        arm=ARm.astype(FP16),
        ucol=uvec.reshape(128, 1).astype(np.float32),
        wkt=np.ascontiguousarray(Wk.T.astype(np.float32)).astype(BF16),
        iota=iota,
        iotab=np.ascontiguousarray(np.tile(iota, (1, Cmax))),
    )

    in_maps = []
    perm = np.full((NCORES, GC), -1, dtype=np.int64)
    offs = np.concatenate([[0], np.cumsum(C)]).astype(np.int64)
    for c in range(NCORES):
        # packed group order (block-major), with per-group block id and column
        gorder = np.concatenate([core_blocks[c][b][0] for b in range(NBLK)])
        gblk = np.concatenate(
            [np.full(len(core_blocks[c][b][0]), b, dtype=np.int64) for b in range(NBLK)]
        )
        gj = np.concatenate(
            [np.arange(len(core_blocks[c][b][0]), dtype=np.int64) for b in range(NBLK)]
        )
        lens = counts[gorder]
        ne = int(lens.sum())
        # element global indices = concatenated ranges cum[g]:cum[g+1]
        lens_cum = np.concatenate([[0], np.cumsum(lens)[:-1]]).astype(np.int64)
        within = np.arange(ne, dtype=np.int64) - np.repeat(lens_cum, lens)
        eidx = np.repeat(cum[gorder], lens) + within
        eblk = np.repeat(gblk, lens)
        ej = np.repeat(gj, lens)
        # position within block (elements are in block-major order)
        blk_sizes = np.bincount(eblk, minlength=NBLK).astype(np.int64)
        blk_start = np.concatenate([[0], np.cumsum(blk_sizes)[:-1]]).astype(np.int64)
        pos = np.arange(ne, dtype=np.int64) - np.repeat(blk_start, blk_sizes)
        t = pos // 128
        i = pos % 128
        Tg = offs[eblk] + t          # global tile index in [0, tot)

        embp3 = np.zeros((128, tot, 129), dtype=BF16)
        embp3[:, :, 128] = BF16(1.0)
        embp3[i, Tg, :128] = emb_bf[eidx]
        embT3 = np.zeros((128, tot, 128), dtype=FP16)
        embT3[:, Tg, i] = emb[eidx].T.astype(FP16)
        segrel = np.full((128, tot), -1.0, dtype=np.float32)
        segrel[i, Tg] = ej.astype(np.float32)
        embLT = np.zeros((128, GC), dtype=FP16)
        embLT[:, gblk * 128 + gj] = emb[last_idx[gorder]].T.astype(FP16)
        perm[c, gblk * 128 + gj] = gorder

        m = dict(
            embp=np.ascontiguousarray(embp3.reshape(128, tot * 129)),
            embTp=np.ascontiguousarray(embT3.reshape(128, tot * 128)),
            segrel=np.ascontiguousarray(segrel),
            embLT=np.ascontiguousarray(embLT),
        )
        m.update(consts)
        in_maps.append(m)
    return C, in_maps, perm


def kernel(embeddings, seg_ids, Wq, bq, Wk, bk):
    global LAST_EXEC_NS, LAST_RESULTS
    Wq = np.asarray(Wq, dtype=np.float32)
    bq = np.asarray(bq, dtype=np.float32)
    Wk = np.asarray(Wk, dtype=np.float32)
    bk = np.asarray(bk, dtype=np.float32)
    embeddings = np.asarray(embeddings)
    seg_ids = np.asarray(seg_ids)

    C, in_maps, perm = _host_prep(embeddings, seg_ids, Wq, bq, Wk, bk)

    key = tuple(C)
    if key not in _cache:
        _cache[key] = _build_program(C)
    nc = _cache[key]

    trace = bool(int(os.environ.get("BASS_KERNEL_TRACE", "0")))
    res = run_bass_kernel_spmd(nc, in_maps, core_ids=list(range(NCORES)), trace=trace)
    LAST_RESULTS = res
    LAST_EXEC_NS = res.exec_time_ns

    out = np.empty((G, D), dtype=np.float32)
    for c in range(NCORES):
        oT = res.results[c]["outT"].astype(np.float32)     # [128, GC]
        dn = res.results[c]["dens"].T.reshape(-1)          # [128, NBLK] -> slot b*128+j
        valid = perm[c] >= 0
        out[perm[c, valid]] = oT[:, valid].T / dn[valid, None] + bk
    return out
